# revision 6
# baseline (speedup 1.0000x reference)
"""DenseGrid multi-resolution 1-D linear interpolation on 8 Trainium2 cores.

Math: out[n, l, f] = (1-fr)*storage[off_l + i0, f] + fr*storage[off_l + i0 + 1, f]
with i0 = floor(x[n]*(R_l-1)), fr = frac(x[n]*(R_l-1)).

Device algorithm (per core, data-parallel over N):
  The whole lookup+lerp is one matmul against "tent" (hat) basis values:
      out[ch=(l,f), n] = sum_{l,j} tent(m_l*x_n - j) * storage[off_l + j, f]
  where tent(v) = relu(1 - |v|) and m_l = R_l - 1.
  1. PE:  psA[(l,j)-row, n] = m_l * x_n           (K=1 outer-product matmul)
  2. DVE: T = tent(psA - j)  via one fused custom DVE op, fp16 output
  3. PE:  psO[n-part, ch]   = T.T @ table         (K=320 over 3 chunks of 128)
  4. DVE: psO(hi) + psO(lo) -> SBUF, DMA out (n-major rows, contiguous)
Tables are host-side layout prep of the tiny (320x4) storage tensor,
replicated to all cores (data-parallel sharding over points).
"""

import math
import numpy as np
import ml_dtypes

import concourse.bass as bass
import concourse.bacc as bacc
import concourse.mybir as mybir
import concourse.tile as tile
from concourse.bass_utils import run_bass_kernel_spmd

# ----------------------------------------------------------------------------
# Problem constants (hardcoded per spec)
# ----------------------------------------------------------------------------
N_FULL = 1_048_576
LEVELS = 16
FEAT = 4
N_CORES = 8
NCP = N_FULL // N_CORES            # points per core = 131072
P = 128                            # SBUF partitions
IP = NCP // P                      # i-slots per partition = 1024
RESOLUTIONS = [2 * i + 1 for i in range(2, LEVELS + 2)]   # [5,7,...,35]
KROWS = sum(RESOLUTIONS)           # 320 tent rows
KPAD = 384                         # padded to 3 chunks of 128
KCH = KPAD // P                    # 3 contract chunks

CHUNK = 512                        # points per inner chunk (fp32 moving max)
GI = CHUNK // P                    # i-slots per chunk = 4
SUPER_I = 64                       # i-slots per super-chunk (output DMA batch)
USE_LO = True                      # hi+lo fp16 split of the value table

# ----------------------------------------------------------------------------
# Custom DVE op: tent(v - j) = relu(1 - |v - j|), j per-partition scalar
# ----------------------------------------------------------------------------
_TENT_NAME = "TENT_ANT_DG"


def _register_tent_op():
    from concourse import dve_ops
    from concourse.dve_spec import Spec, Src0, C0, One, relu, maxx, lower
    from concourse.dve_table_gen import DveOpSpec

    if any(op.name == _TENT_NAME for op in dve_ops.OPS):
        return next(op for op in dve_ops.OPS if op.name == _TENT_NAME)

    body = relu(One - maxx(Src0 - C0, C0 - Src0))
    spec = Spec(
        body=body,
        reference=lambda in0, in1, s0, s1, imm2: np.maximum(
            1.0 - np.abs(np.asarray(in0, np.float32) - s0), 0.0
        ),
    )
    shas = {}
    for ver in ("v3", "v4"):
        s = DveOpSpec(name=_TENT_NAME, opcode=0, uops=lower(spec, ver=ver), rd1_en=False)
        shas[ver] = s.sha(ver)
    op = dve_ops.DveOp(_TENT_NAME, spec, subdim=False, uops_sha=shas)
    dve_ops.OPS.append(op)
    dve_ops._SUB_OPCODE_FOR_NAME[op.name] = (
        dve_ops._CUSTOM_DVE_ROW_BASE + len(dve_ops.OPS) - 1
    )
    dve_ops.CUSTOM_DVE_SPECS[op.name] = op.spec
    return op


# ----------------------------------------------------------------------------
# Host table prep (tiny: 320x4 -> packed SBUF layouts; pure layout/dtype work)
# ----------------------------------------------------------------------------
def make_tables(storage, resolutions):
    storage = np.asarray(storage, np.float32)
    res = np.asarray(resolutions, np.int64)
    offs = np.concatenate([[0], np.cumsum(res)[:-1]])
    row_m = np.zeros(KPAD, np.float32)
    row_j = np.full(KPAD, 2.0, np.float32)      # pad rows: tent(0*x-2) = 0
    mvals = np.zeros((KPAD, FEAT * LEVELS), np.float32)   # [krow, ch]
    r = 0
    for l in range(LEVELS):
        m = int(res[l]) - 1
        for j in range(int(res[l])):
            row_m[r] = m
            row_j[r] = j
            mvals[r, 4 * l : 4 * l + 4] = storage[offs[l] + j]
            r += 1
    assert r == KROWS

    hi = mvals.astype(ml_dtypes.bfloat16 if False else np.float16)
    lo = (mvals - hi.astype(np.float32)).astype(np.float16)

    mstat = row_m.reshape(1, KPAD)                                   # [1, 384]
    jvec = row_j.reshape(KCH, P).T.copy()                            # [128, 3]
    # mv[r_local, k*128 + c]: c<64 -> hi ch=c ; c>=64 -> lo ch=c-64
    ncols = 128 if USE_LO else 64
    mv = np.zeros((P, KCH * ncols), np.float16)
    for k in range(KCH):
        mv[:, k * ncols : k * ncols + 64] = hi[k * P : (k + 1) * P]
        if USE_LO:
            mv[:, k * ncols + 64 : (k + 1) * ncols] = lo[k * P : (k + 1) * P]
    return mstat, jvec, mv


# ----------------------------------------------------------------------------
# Bass program (SPMD, one program for all cores)
# ----------------------------------------------------------------------------
def build_program(ncp=NCP):
    tent_op = _register_tent_op()
    ip = ncp // P                       # i-slots
    n_super = max(1, ip // SUPER_I)
    super_i = ip // n_super             # i-slots per super-chunk
    chunks_per_super = super_i // GI
    sup_pts = super_i * P               # points per super-chunk

    f32 = mybir.dt.float32
    f16 = mybir.dt.float16
    ncols = 128 if USE_LO else 64

    nc = bacc.Bacc()
    x_ext = nc.declare_dram_parameter("x", [ncp], f32, isOutput=False)
    mstat_ext = nc.declare_dram_parameter("mstat", [1, KPAD], f32, isOutput=False)
    jvec_ext = nc.declare_dram_parameter("jvec", [P, KCH], f32, isOutput=False)
    mv_ext = nc.declare_dram_parameter("mv", [P, KCH * ncols], f16, isOutput=False)
    out_ext = nc.declare_dram_parameter("out", [P, ip, 64], f32, isOutput=True)

    with tile.TileContext(nc) as tc:
        with (
            tc.tile_pool(name="consts", bufs=1) as cpool,
            tc.tile_pool(name="xin", bufs=2) as xpool,
            tc.tile_pool(name="tent", bufs=2) as tpool,
            tc.tile_pool(name="obuf", bufs=2) as opool,
            tc.tile_pool(name="psA", bufs=1, space="PSUM") as psa_pool,
            tc.tile_pool(name="psO", bufs=3, space="PSUM") as pso_pool,
            tc.tile_pool(name="psW", bufs=1, space="PSUM") as psw_pool,
        ):
            # PE Matmult/Ldweights can carry only ONE sync-wait in codegen.
            # Every matmul must therefore see at most one new foreign-engine
            # tick. After each DMA that feeds a PE operand, a 1x1 "observer"
            # matmul absorbs the DMA wait so the real matmuls never pair a
            # DMA wait with a compute-engine wait.
            warm_t = psw_pool.tile([1, 8], f32, tag="warm")

            def observe(t):
                nc.tensor.matmul(
                    warm_t[0:1, 0:1],
                    lhsT=t[0:1, 0:1],
                    rhs=t[0:1, 0:1],
                    start=True,
                    stop=True,
                )

            mstat_t = cpool.tile([1, KPAD], f32, tag="mstat")
            jvec_t = cpool.tile([P, KCH], f32, tag="jvec")
            mv_t = cpool.tile([P, KCH * ncols], f16, tag="mv")
            nc.sync.dma_start(out=mstat_t[:], in_=mstat_ext[:])
            nc.sync.dma_start(out=jvec_t[:], in_=jvec_ext[:])
            nc.sync.dma_start(out=mv_t[:], in_=mv_ext[:])
            observe(mstat_t)
            observe(mv_t)

            for s in range(n_super):
                x_t = xpool.tile([1, sup_pts], f32, tag="x")
                nc.sync.dma_start(
                    out=x_t[:], in_=x_ext[s * sup_pts : (s + 1) * sup_pts]
                )
                observe(x_t)
                o_t = opool.tile([P, super_i * 64], f32, tag="o")
                for cl in range(chunks_per_super):
                    xs = x_t[0:1, cl * CHUNK : (cl + 1) * CHUNK]
                    psA = [
                        psa_pool.tile([P, CHUNK], f32, tag=f"A{k}", name=f"psA{k}_{s}_{cl}")
                        for k in range(KCH)
                    ]
                    T = [
                        tpool.tile([P, CHUNK], f16, tag=f"T{k}", name=f"T{k}_{s}_{cl}")
                        for k in range(KCH)
                    ]
                    for k in range(KCH):
                        nc.tensor.matmul(
                            psA[k][:],
                            lhsT=mstat_t[0:1, k * P : (k + 1) * P],
                            rhs=xs,
                            start=True,
                            stop=True,
                        )
                        nc.vector._custom_dve(
                            tent_op,
                            out=T[k][:],
                            in0=psA[k][:],
                            s0=jvec_t[:, k : k + 1],
                        )
                    # hi and lo table halves accumulate into the same PSUM
                    # columns (PSUM may only feed one input downstream).
                    psO = pso_pool.tile([P, GI * 64], f32, tag="O")
                    n_acc = 2 * KCH if USE_LO else KCH
                    for g in range(GI):
                        a = 0
                        for k in range(KCH):
                            for half in range(2 if USE_LO else 1):
                                nc.tensor.matmul(
                                    psO[:, g * 64 : (g + 1) * 64],
                                    lhsT=T[k][:, g * P : (g + 1) * P],
                                    rhs=mv_t[
                                        :,
                                        k * ncols + half * 64 : k * ncols
                                        + half * 64
                                        + 64,
                                    ],
                                    start=(a == 0),
                                    stop=(a == n_acc - 1),
                                )
                                a += 1
                    # copy on the DVE so the next chunk's matmuls see the
                    # psO release and the T tiles on one semaphore (DVE)
                    oc = cl * GI * 64
                    nc.vector.tensor_copy(o_t[:, oc : oc + GI * 64], psO[:])
                nc.sync.dma_start(
                    out=out_ext[:, s * super_i : (s + 1) * super_i, :],
                    in_=o_t[:],
                )
    nc.finalize()
    return nc


# ----------------------------------------------------------------------------
# Host entry point
# ----------------------------------------------------------------------------
def _proc_order(x_shard):
    """Permute points into the device processing order n' = c*512 + g*128 + q,
    where point = q*IP + c*GI + g (pure layout transform)."""
    ncp = x_shard.shape[0]
    ip = ncp // P
    return np.ascontiguousarray(
        x_shard.reshape(P, ip // GI, GI).transpose(1, 2, 0)
    ).reshape(-1)


_PROGRAM_CACHE = {}


def kernel(x, storage, resolutions):
    x = np.asarray(x, np.float32).reshape(-1)
    assert x.shape[0] == N_FULL
    mstat, jvec, mv = make_tables(storage, resolutions)

    if NCP not in _PROGRAM_CACHE:
        _PROGRAM_CACHE[NCP] = build_program(NCP)
    nc = _PROGRAM_CACHE[NCP]

    in_maps = []
    for c in range(N_CORES):
        shard = x[c * NCP : (c + 1) * NCP]
        in_maps.append(
            {
                "x": _proc_order(shard),
                "mstat": mstat,
                "jvec": jvec,
                "mv": mv,
            }
        )
    res = run_bass_kernel_spmd(nc, in_maps, list(range(N_CORES)))
    outs = [r["out"].reshape(NCP, LEVELS, FEAT) for r in res.results]
    return np.concatenate(outs, axis=0)


# revision 8
# speedup vs baseline: 3.3895x; 3.3895x over previous
"""DenseGrid multi-resolution 1-D linear interpolation on 8 Trainium2 cores.

Math: out[n, l, f] = (1-fr)*storage[off_l + i0, f] + fr*storage[off_l + i0 + 1, f]
with i0 = floor(x[n]*(R_l-1)), fr = frac(x[n]*(R_l-1)).

Device algorithm (per core, data-parallel over N):
  The whole lookup+lerp is one matmul against "tent" (hat) basis values:
      out[ch=(l,f), n] = sum_{l,j} tent(m_l*x_n - j) * storage[off_l + j, f]
  where tent(v) = relu(1 - |v|) and m_l = R_l - 1.
  1. PE:  psA[(l,j)-row, n] = m_l*(xh_n + xl_n)   (K=2 fp16 matmul; xh/xl is a
          lossless hi/lo split of fp32 x, so psA is exact to ~2^-23; the three
          128-row chunks run concurrently via PE row tiling)
  2. DVE (k=0,1) / ACT (k=2): T = tent(psA - j), fp16 output
  3. PE:  psO[n-part, ch] = T.T @ table           (K=320 over 3 chunks of 128)
  4. DVE+ACT: psO -> SBUF (split), DMA out (n-major rows, contiguous)
Tables are host-side layout prep of the tiny (320x4) storage tensor,
replicated to all cores (data-parallel sharding over points).
"""

import numpy as np

import concourse.bacc as bacc
import concourse.mybir as mybir
import concourse.tile as tile
from concourse.bass_utils import run_bass_kernel_spmd

# ----------------------------------------------------------------------------
# Problem constants (hardcoded per spec)
# ----------------------------------------------------------------------------
N_FULL = 1_048_576
LEVELS = 16
FEAT = 4
N_CORES = 8
NCP = N_FULL // N_CORES            # points per core = 131072
P = 128                            # SBUF partitions
IP = NCP // P                      # i-slots per partition = 1024
RESOLUTIONS = [2 * i + 1 for i in range(2, LEVELS + 2)]   # [5,7,...,35]
KROWS = sum(RESOLUTIONS)           # 320 tent rows
KPAD = 384                         # padded to 3 chunks of 128
KCH = KPAD // P                    # 3 contract chunks

CHUNK = 1024                       # points per inner chunk (fp16 moving max)
GI = CHUNK // P                    # i-slots (128-pt groups) per chunk = 8
SUPER_I = 64                       # i-slots per super-chunk (output DMA batch)

# ----------------------------------------------------------------------------
# Custom DVE op: tent(v - j) = relu(1 - |v - j|), j per-partition scalar
# ----------------------------------------------------------------------------
_TENT_NAME = "TENT_ANT_DG"


def _register_tent_op():
    from concourse import dve_ops
    from concourse.dve_spec import Spec, Src0, C0, One, relu, maxx, lower
    from concourse.dve_table_gen import DveOpSpec

    if any(op.name == _TENT_NAME for op in dve_ops.OPS):
        return next(op for op in dve_ops.OPS if op.name == _TENT_NAME)

    body = relu(One - maxx(Src0 - C0, C0 - Src0))
    spec = Spec(
        body=body,
        reference=lambda in0, in1, s0, s1, imm2: np.maximum(
            1.0 - np.abs(np.asarray(in0, np.float32) - s0), 0.0
        ),
    )
    shas = {}
    for ver in ("v3", "v4"):
        s = DveOpSpec(name=_TENT_NAME, opcode=0, uops=lower(spec, ver=ver), rd1_en=False)
        shas[ver] = s.sha(ver)
    op = dve_ops.DveOp(_TENT_NAME, spec, subdim=False, uops_sha=shas)
    dve_ops.OPS.append(op)
    dve_ops._SUB_OPCODE_FOR_NAME[op.name] = (
        dve_ops._CUSTOM_DVE_ROW_BASE + len(dve_ops.OPS) - 1
    )
    dve_ops.CUSTOM_DVE_SPECS[op.name] = op.spec
    return op


# ----------------------------------------------------------------------------
# Host table prep (tiny: 320x4 -> packed SBUF layouts; pure layout/dtype work)
# ----------------------------------------------------------------------------
def make_tables(storage, resolutions):
    storage = np.asarray(storage, np.float32)
    res = np.asarray(resolutions, np.int64)
    offs = np.concatenate([[0], np.cumsum(res)[:-1]])
    row_m = np.zeros(KPAD, np.float32)
    row_j = np.full(KPAD, 2.0, np.float32)      # pad rows: tent(0*x-2) = 0
    mvals = np.zeros((KPAD, FEAT * LEVELS), np.float32)   # [krow, ch]
    r = 0
    for l in range(LEVELS):
        m = int(res[l]) - 1
        for j in range(int(res[l])):
            row_m[r] = m
            row_j[r] = j
            mvals[r, 4 * l : 4 * l + 4] = storage[offs[l] + j]
            r += 1
    assert r == KROWS

    # affine stationary: rows (32k, 32k+1) hold m for K-chunk k (xh and xl
    # share the same coefficient)
    mstat = np.zeros((P, P), np.float16)
    for k in range(KCH):
        mstat[32 * k, :] = row_m[k * P : (k + 1) * P]
        mstat[32 * k + 1, :] = row_m[k * P : (k + 1) * P]
    jvec = row_j.reshape(KCH, P).T.copy()                  # [128, 3] f32
    njvec = (-row_j).reshape(KCH, P).T.copy()              # [128, 3] f32
    mv = np.zeros((P, KCH * 64), np.float16)               # [r_local, k*64+ch]
    for k in range(KCH):
        mv[:, k * 64 : (k + 1) * 64] = mvals[k * P : (k + 1) * P].astype(np.float16)
    return mstat, jvec, njvec, mv


# ----------------------------------------------------------------------------
# Bass program (SPMD, one program for all cores)
# ----------------------------------------------------------------------------
def build_program(ncp=NCP):
    tent_op = _register_tent_op()
    ip = ncp // P                       # i-slots
    n_super = max(1, ip // SUPER_I)
    super_i = ip // n_super             # i-slots per super-chunk
    chunks_per_super = super_i // GI
    sup_pts = super_i * P               # points per super-chunk

    f32 = mybir.dt.float32
    f16 = mybir.dt.float16
    AF = mybir.ActivationFunctionType

    nc = bacc.Bacc()
    x_ext = nc.declare_dram_parameter("x", [2, ncp], f16, isOutput=False)
    mstat_ext = nc.declare_dram_parameter("mstat", [P, P], f16, isOutput=False)
    jvec_ext = nc.declare_dram_parameter("jvec", [P, KCH], f32, isOutput=False)
    njvec_ext = nc.declare_dram_parameter("njvec", [P, KCH], f32, isOutput=False)
    mv_ext = nc.declare_dram_parameter("mv", [P, KCH * 64], f16, isOutput=False)
    out_ext = nc.declare_dram_parameter("out", [P, ip, 64], f32, isOutput=True)

    with tile.TileContext(nc) as tc:
        with (
            tc.tile_pool(name="consts", bufs=1) as cpool,
            tc.tile_pool(name="xin", bufs=2) as xpool,
            tc.tile_pool(name="tent", bufs=2) as tpool,
            tc.tile_pool(name="absb", bufs=2) as apool,
            tc.tile_pool(name="obuf", bufs=2) as opool,
            tc.tile_pool(name="psA", bufs=1, space="PSUM") as psa_pool,
            tc.tile_pool(name="psO", bufs=2, space="PSUM") as pso_pool,
        ):
            mstat_t = cpool.tile([P, P], f16, tag="mstat")
            jvec_t = cpool.tile([P, KCH], f32, tag="jvec")
            njvec_t = cpool.tile([P, KCH], f32, tag="njvec")
            mv_t = cpool.tile([P, KCH * 64], f16, tag="mv")
            nc.sync.dma_start(out=mstat_t[:], in_=mstat_ext[:])
            nc.sync.dma_start(out=jvec_t[:], in_=jvec_ext[:])
            nc.sync.dma_start(out=njvec_t[:], in_=njvec_ext[:])
            nc.sync.dma_start(out=mv_t[:], in_=mv_ext[:])

            for s in range(n_super):
                x_t = xpool.tile([66, sup_pts], f16, tag="x", name=f"x_{s}")
                for k in range(KCH):
                    nc.sync.dma_start(
                        out=x_t[32 * k : 32 * k + 2, :],
                        in_=x_ext[:, s * sup_pts : (s + 1) * sup_pts],
                    )
                o_t = opool.tile([P, super_i * 64], f32, tag="o", name=f"o_{s}")
                for cl in range(chunks_per_super):
                    psA = [
                        psa_pool.tile([P, CHUNK], f32, tag=f"A{k}", name=f"psA{k}_{s}_{cl}")
                        for k in range(KCH)
                    ]
                    T = [
                        tpool.tile([P, CHUNK], f16, tag=f"T{k}", name=f"T{k}_{s}_{cl}")
                        for k in range(KCH)
                    ]
                    for k in range(KCH):
                        for h in range(2):  # fp32 PSUM: max 512 cols per matmul
                            xs = slice(cl * CHUNK + h * 512, cl * CHUNK + (h + 1) * 512)
                            nc.tensor.matmul(
                                psA[k][:, h * 512 : (h + 1) * 512],
                                lhsT=mstat_t[32 * k : 32 * k + 2, :],
                                rhs=x_t[32 * k : 32 * k + 2, xs],
                                start=True,
                                stop=True,
                                tile_position=(32 * k, 0),
                            )
                    # tent nonlinearity: k=0,1 on DVE (fused custom op),
                    # k=2 on ACT (Abs then Relu) to balance engine load
                    for k in range(2):
                        nc.vector._custom_dve(
                            tent_op,
                            out=T[k][:],
                            in0=psA[k][:],
                            s0=jvec_t[:, k : k + 1],
                        )
                    abs_t = apool.tile([P, CHUNK], f32, tag="abs", name=f"abs_{s}_{cl}")
                    nc.scalar.activation(
                        abs_t[:], psA[2][:], AF.Abs, bias=njvec_t[:, 2:3], scale=1.0
                    )
                    nc.scalar.activation(
                        T[2][:], abs_t[:], AF.Relu, bias=1.0, scale=-1.0
                    )
                    psO = pso_pool.tile([P, GI * 64], f32, tag="O", name=f"psO_{s}_{cl}")
                    for g in range(GI):
                        for k in range(KCH):
                            nc.tensor.matmul(
                                psO[:, g * 64 : (g + 1) * 64],
                                lhsT=T[k][:, g * P : (g + 1) * P],
                                rhs=mv_t[:, k * 64 : (k + 1) * 64],
                                start=(k == 0),
                                stop=(k == KCH - 1),
                            )
                    half = GI * 32
                    oc = cl * GI * 64
                    nc.vector.tensor_copy(o_t[:, oc : oc + half], psO[:, 0:half])
                    nc.scalar.copy(o_t[:, oc + half : oc + 2 * half], psO[:, half:])
                nc.sync.dma_start(
                    out=out_ext[:, s * super_i : (s + 1) * super_i, :],
                    in_=o_t[:],
                )
    nc.finalize()
    return nc


# ----------------------------------------------------------------------------
# Host entry point
# ----------------------------------------------------------------------------
def _proc_order(x_shard):
    """Permute points into the device processing order n' = c*CHUNK + g*128 + q
    (point = q*IP + c*GI + g), then split fp32 x losslessly into an fp16
    (hi, lo) pair for the PE's fp16 datapath. Pure layout/precision prep."""
    ncp = x_shard.shape[0]
    ip = ncp // P
    xp = np.ascontiguousarray(
        x_shard.reshape(P, ip // GI, GI).transpose(1, 2, 0)
    ).reshape(-1)
    xh = xp.astype(np.float16)
    xl = (xp - xh.astype(np.float32)).astype(np.float16)
    return np.stack([xh, xl])


_PROGRAM_CACHE = {}


def kernel(x, storage, resolutions):
    x = np.asarray(x, np.float32).reshape(-1)
    assert x.shape[0] == N_FULL
    mstat, jvec, njvec, mv = make_tables(storage, resolutions)

    if NCP not in _PROGRAM_CACHE:
        _PROGRAM_CACHE[NCP] = build_program(NCP)
    nc = _PROGRAM_CACHE[NCP]

    in_maps = []
    for c in range(N_CORES):
        shard = x[c * NCP : (c + 1) * NCP]
        in_maps.append(
            {
                "x": _proc_order(shard),
                "mstat": mstat,
                "jvec": jvec,
                "njvec": njvec,
                "mv": mv,
            }
        )
    res = run_bass_kernel_spmd(nc, in_maps, list(range(N_CORES)))
    outs = [r["out"].reshape(NCP, LEVELS, FEAT) for r in res.results]
    return np.concatenate(outs, axis=0)
